# revision 4
# baseline (speedup 1.0000x reference)
"""Trainium2 Bass kernel for nn_CustomLoss (argmax-distance weighted loss).

reference:
    arg = argmax(target, axis=1)              # [B]
    delta = distance[arg]                     # [B]
    err = |distance[None,:] - delta[:,None]| + 1
    loss = sum((output - target) * err) / B

Algorithm (no gathers, data-parallel over 8 NeuronCores):
  With dist = [-0.5, -0.34, 0, 0.34, 0.5] and e_a = [t_a >= max_c t_c]:
    2*delta = (e4 - e0) + 0.68*(e3 - e1)          (dist[2]=0 -> e2 unused;
                                                   argmax==2 gives w2=0=2*dist[2])
    err[b,c] + 1 -> (|2*delta - 2*dist_c| + 2)/2
  loss*2*B = sum over b,c of (o - t) * (|w2 - 2*dist_c| + 2)

Per-core layout: rows on 128 partitions, 5 classes interleaved along free dim,
8 tiles of [128, 2560].  Engines: DMA loads t (f32, HWDGE) + o (bf16 cast,
SWDGE); GPSIMD does the 4-op max tree; VectorE does compares and the fused
(w+2)*d product with per-partition accum; ScalarE does the bf16 cast and the
5 Abs activations.  Output: [128, ntiles] partial sums per core, reduced on
host and divided by 2*B.
"""

from contextlib import ExitStack

import numpy as np

P = 128
C = 5
DIST = (-0.5, -0.34, 0.0, 0.34, 0.5)
B = 4194304
NCORES = 8
ROWS_PER_CORE = B // NCORES  # 524288
G = 512                      # rows per partition per tile
NTILES = ROWS_PER_CORE // (P * G)  # 8

_CACHE = {}


def _build_nc():
    import concourse.bacc as bacc
    import concourse.mybir as mybir
    import concourse.tile as tile

    F32 = mybir.dt.float32
    BF16 = mybir.dt.bfloat16
    FREE = C * G

    nc = bacc.Bacc(target_bir_lowering=False)

    # Register activation-bias constants (-2*dist[c]) in the const-AP database,
    # mirroring what Bass.__init__ does for 0.0/1.0.
    for c in range(C):
        val = -2.0 * DIST[c]
        if (F32, val) not in nc.const_aps.aps:
            tensor = nc.alloc_sbuf_tensor(f"const-f32-bias{c}", [P, 1], F32)
            nc.gpsimd.memset(tensor.ap(), val)
            nc.const_aps.aps[(F32, val)] = tensor.ap()
    nc.all_engine_barrier()

    t_in = nc.declare_dram_parameter("t", [ROWS_PER_CORE, C], F32, isOutput=False)
    o_in = nc.declare_dram_parameter("o", [ROWS_PER_CORE, C], F32, isOutput=False)
    out = nc.declare_dram_parameter("out", [P, NTILES], F32, isOutput=True)

    # row = n*(P*G) + p*G + g ; per-partition data is contiguous in DRAM
    t_tiled = t_in.rearrange("(n p g) c -> n p (g c)", p=P, g=G)
    o_tiled = o_in.rearrange("(n p g) c -> n p (g c)", p=P, g=G)

    with ExitStack() as ctx:
        tc = ctx.enter_context(tile.TileContext(nc))
        pool = ctx.enter_context(tc.tile_pool(name="work", bufs=3))
        accp = ctx.enter_context(tc.tile_pool(name="acc", bufs=1))
        acc = accp.tile([P, NTILES], F32)

        for k in range(NTILES):
            t = pool.tile([P, FREE], F32, tag="t")
            nc.sync.dma_start(t[:, :], t_tiled[k])
            o = pool.tile([P, FREE], BF16, tag="o")
            nc.gpsimd.dma_start(o[:, :], o_tiled[k])  # f32 -> bf16 cast in DMA

            tb = pool.tile([P, FREE], BF16, tag="tb")
            nc.scalar.copy(tb[:, :], t[:, :])  # ACT cast f32->bf16

            d = pool.tile([P, FREE], BF16, tag="d")
            nc.vector.tensor_sub(d[:, :], o[:, :], tb[:, :])

            tv = t[:, :].rearrange("p (g c) -> p g c", c=C)

            # max tree (TT is illegal on GPSIMD at the ISA level on trn2)
            m01 = pool.tile([P, G], F32, tag="m01")
            nc.vector.tensor_max(m01[:, :], tv[:, :, 0], tv[:, :, 1])
            m23 = pool.tile([P, G], F32, tag="m23")
            nc.vector.tensor_max(m23[:, :], tv[:, :, 2], tv[:, :, 3])
            m03 = pool.tile([P, G], F32, tag="m03")
            nc.vector.tensor_max(m03[:, :], m01[:, :], m23[:, :])
            m = pool.tile([P, G], F32, tag="m")
            nc.vector.tensor_max(m[:, :], m03[:, :], tv[:, :, 4])

            # one-hot comparisons (only classes 0,1,3,4 needed)
            e = {}
            for a in (0, 1, 3, 4):
                e[a] = pool.tile([P, G], BF16, tag=f"e{a}", name=f"e{a}")
                nc.vector.tensor_tensor(
                    e[a][:, :], tv[:, :, a], m[:, :], op=mybir.AluOpType.is_ge
                )

            u = pool.tile([P, G], BF16, tag="u")
            nc.vector.tensor_sub(u[:, :], e[4][:, :], e[0][:, :])
            v = pool.tile([P, G], BF16, tag="v")
            nc.vector.tensor_sub(v[:, :], e[3][:, :], e[1][:, :])
            w2 = pool.tile([P, G], BF16, tag="w2")
            # w2 = (v * 0.68) + u
            nc.vector.scalar_tensor_tensor(
                w2[:, :], v[:, :], 0.68, u[:, :],
                mybir.AluOpType.mult, mybir.AluOpType.add,
            )

            # wI[:, g, c] = |w2 - 2*dist[c]|  (ScalarE)
            wI = pool.tile([P, FREE], BF16, tag="wI")
            wIv = wI[:, :].rearrange("p (g c) -> p g c", c=C)
            for c in range(C):
                nc.scalar.activation(
                    wIv[:, :, c], w2[:, :], mybir.ActivationFunctionType.Abs,
                    bias=-2.0 * DIST[c], scale=1.0,
                )

            # J'_k = sum((wI + 2) * d) per partition row
            scratch = pool.tile([P, FREE], BF16, tag="scratch")
            nc.vector.scalar_tensor_tensor(
                scratch[:, :], wI[:, :], 2.0, d[:, :],
                mybir.AluOpType.add, mybir.AluOpType.mult,
                accum_out=acc[:, k : k + 1],
            )

        nc.sync.dma_start(out[:, :], acc[:, :])
    nc.finalize()
    return nc


def _get_nc():
    if "nc" not in _CACHE:
        _CACHE["nc"] = _build_nc()
    return _CACHE["nc"]


def kernel(output, target, distance, _want_results=False):
    from concourse.bass_utils import run_bass_kernel_spmd

    output = np.asarray(output, dtype=np.float32)
    target = np.asarray(target, dtype=np.float32)
    distance = np.asarray(distance, dtype=np.float32)
    assert output.shape == (B, C) and target.shape == (B, C)
    assert np.allclose(distance, np.asarray(DIST, np.float32)), distance

    nc = _get_nc()
    o_sh = output.reshape(NCORES, ROWS_PER_CORE, C)
    t_sh = target.reshape(NCORES, ROWS_PER_CORE, C)
    in_maps = [
        {"t": np.ascontiguousarray(t_sh[i]), "o": np.ascontiguousarray(o_sh[i])}
        for i in range(NCORES)
    ]
    res = run_bass_kernel_spmd(nc, in_maps, core_ids=list(range(NCORES)))
    total = 0.0
    for r in res.results:
        total += float(r["out"].astype(np.float64).sum())
    loss = np.float32(total / 2.0 / B)
    if _want_results:
        return loss, res
    return loss


# revision 9
# speedup vs baseline: 1.0743x; 1.0743x over previous
"""Trainium2 Bass kernel for nn_CustomLoss (argmax-distance weighted loss).

reference:
    arg = argmax(target, axis=1)              # [B]
    delta = distance[arg]                     # [B]
    err = |distance[None,:] - delta[:,None]| + 1
    loss = sum((output - target) * err) / B

Algorithm (no gathers, data-parallel over 8 NeuronCores):
  With dist = [-0.5, -0.34, 0, 0.34, 0.5] and e_a = [t_a >= max_c t_c]:
    2*delta = (e4 - e0) + 0.68*(e3 - e1)          (dist[2]=0 -> e2 unused;
                                                   argmax==2 gives w2=0=2*dist[2])
    err[b,c] + 1 -> (|2*delta - 2*dist_c| + 2)/2
  loss*2*B = sum over b,c of (o - t) * (|w2 - 2*dist_c| + 2)

Per-core layout: rows on 128 partitions, 5 classes interleaved along free dim,
8 tiles of [128, 2560].  Engines: DMA loads t (f32, HWDGE) + o (bf16 cast,
SWDGE); GPSIMD does the 4-op max tree; VectorE does compares and the fused
(w+2)*d product with per-partition accum; ScalarE does the bf16 cast and the
5 Abs activations.  Output: [128, ntiles] partial sums per core, reduced on
host and divided by 2*B.
"""

from contextlib import ExitStack

import numpy as np

P = 128
C = 5
DIST = (-0.5, -0.34, 0.0, 0.34, 0.5)
B = 4194304
NCORES = 8
ROWS_PER_CORE = B // NCORES  # 524288
G = 512                      # rows per partition per tile
NTILES = ROWS_PER_CORE // (P * G)  # 8

_CACHE = {}


def _build_nc():
    import concourse.bacc as bacc
    import concourse.mybir as mybir
    import concourse.tile as tile

    F32 = mybir.dt.float32
    BF16 = mybir.dt.bfloat16
    FREE = C * G

    nc = bacc.Bacc(target_bir_lowering=False)

    # Register activation-bias constants (-2*dist[c]) in the const-AP database,
    # mirroring what Bass.__init__ does for 0.0/1.0.
    for c in range(C):
        val = -2.0 * DIST[c]
        if (F32, val) not in nc.const_aps.aps:
            tensor = nc.alloc_sbuf_tensor(f"const-f32-bias{c}", [P, 1], F32)
            nc.gpsimd.memset(tensor.ap(), val)
            nc.const_aps.aps[(F32, val)] = tensor.ap()
    nc.all_engine_barrier()

    t_in = nc.declare_dram_parameter("t", [ROWS_PER_CORE, C], F32, isOutput=False)
    o_in = nc.declare_dram_parameter("o", [ROWS_PER_CORE, C], F32, isOutput=False)
    out = nc.declare_dram_parameter("out", [1, 2 * G], F32, isOutput=True)

    # row = n*(P*G) + p*G + g ; per-partition data is contiguous in DRAM
    t_tiled = t_in.rearrange("(n p g) c -> n p (g c)", p=P, g=G)
    o_tiled = o_in.rearrange("(n p g) c -> n p (g c)", p=P, g=G)

    ones_bf16 = nc.const_aps.aps[(BF16, 1.0)]  # [128, 1] of 1.0, preregistered

    with ExitStack() as ctx:
        tc = ctx.enter_context(tile.TileContext(nc))
        pool = ctx.enter_context(tc.tile_pool(name="work", bufs=3))
        psp = ctx.enter_context(tc.tile_pool(name="ps", bufs=1, space="PSUM"))
        outp = ctx.enter_context(tc.tile_pool(name="outp", bufs=1))
        ps_p = psp.tile([1, G], F32)   # sum of wI*d
        ps_d = psp.tile([1, G], F32)   # sum of d

        for k in range(NTILES):
            t = pool.tile([P, FREE], F32, tag="t")
            nc.sync.dma_start(t[:, :], t_tiled[k])
            o = pool.tile([P, FREE], BF16, tag="o")
            nc.gpsimd.dma_start(o[:, :], o_tiled[k])  # f32 -> bf16 cast in DMA

            tb = pool.tile([P, FREE], BF16, tag="tb")
            nc.scalar.copy(tb[:, :], t[:, :])  # ACT cast f32->bf16

            d = pool.tile([P, FREE], BF16, tag="d")
            nc.vector.tensor_sub(d[:, :], o[:, :], tb[:, :])

            tv = t[:, :].rearrange("p (g c) -> p g c", c=C)

            # m[p,g] = max over the 5 classes (segmented reduce, unit-stride)
            m = pool.tile([P, G], F32, tag="m")
            nc.vector.tensor_reduce(
                m[:, :], tv, axis=mybir.AxisListType.X, op=mybir.AluOpType.max
            )

            # E[p,g,c] = [t >= m]  (one pass, m broadcast along class dim)
            E = pool.tile([P, FREE], BF16, tag="E")
            nc.vector.tensor_tensor(
                E[:, :].rearrange("p (g c) -> p g c", c=C),
                tv,
                m[:, :].to_broadcast([P, G, C]),
                op=mybir.AluOpType.is_ge,
            )

            Ev = E[:, :].rearrange("p (g c) -> p g c", c=C)
            u = pool.tile([P, G], BF16, tag="u")
            nc.vector.tensor_sub(u[:, :], Ev[:, :, 4], Ev[:, :, 0])
            v = pool.tile([P, G], BF16, tag="v")
            nc.vector.tensor_sub(v[:, :], Ev[:, :, 3], Ev[:, :, 1])
            w2 = pool.tile([P, G], BF16, tag="w2")
            # w2 = (v * 0.68) + u  == 2*delta
            nc.vector.scalar_tensor_tensor(
                w2[:, :], v[:, :], 0.68, u[:, :],
                mybir.AluOpType.mult, mybir.AluOpType.add,
            )

            # wI[:, g, c] = |w2 - 2*dist[c]|  (ScalarE)
            wI = pool.tile([P, FREE], BF16, tag="wI")
            wIv = wI[:, :].rearrange("p (g c) -> p g c", c=C)
            for c in range(C):
                nc.scalar.activation(
                    wIv[:, :, c], w2[:, :], mybir.ActivationFunctionType.Abs,
                    bias=-2.0 * DIST[c], scale=1.0,
                )

            # p = wI * d  (bf16 2x mode), then TensorE ones-matmul reduces
            # p and d into PSUM accumulators across all tiles
            p = pool.tile([P, FREE], BF16, tag="p")
            nc.vector.tensor_mul(p[:, :], wI[:, :], d[:, :])
            for j in range(C):
                first = k == 0 and j == 0
                last = k == NTILES - 1 and j == C - 1
                nc.tensor.matmul(
                    ps_p[:, :], ones_bf16, p[:, j * G : (j + 1) * G],
                    start=first, stop=last,
                )
                nc.tensor.matmul(
                    ps_d[:, :], ones_bf16, d[:, j * G : (j + 1) * G],
                    start=first, stop=last,
                )

        # readout: [1, 2G] f32 -> DRAM; host computes (sum0 + 2*sum1) / 2B
        res = outp.tile([1, 2 * G], F32)
        nc.scalar.copy(res[:, 0:G], ps_p[:, :])
        nc.scalar.copy(res[:, G : 2 * G], ps_d[:, :])
        nc.sync.dma_start(out[:, :], res[:, :])
    nc.finalize()
    return nc


def _get_nc():
    if "nc" not in _CACHE:
        _CACHE["nc"] = _build_nc()
    return _CACHE["nc"]


def kernel(output, target, distance, _want_results=False):
    from concourse.bass_utils import run_bass_kernel_spmd

    output = np.asarray(output, dtype=np.float32)
    target = np.asarray(target, dtype=np.float32)
    distance = np.asarray(distance, dtype=np.float32)
    assert output.shape == (B, C) and target.shape == (B, C)
    assert np.allclose(distance, np.asarray(DIST, np.float32)), distance

    nc = _get_nc()
    o_sh = output.reshape(NCORES, ROWS_PER_CORE, C)
    t_sh = target.reshape(NCORES, ROWS_PER_CORE, C)
    in_maps = [
        {"t": np.ascontiguousarray(t_sh[i]), "o": np.ascontiguousarray(o_sh[i])}
        for i in range(NCORES)
    ]
    res = run_bass_kernel_spmd(nc, in_maps, core_ids=list(range(NCORES)))
    total = 0.0
    for r in res.results:
        arr = r["out"].astype(np.float64).reshape(2, G)
        total += float(arr[0].sum() + 2.0 * arr[1].sum())
    loss = np.float32(total / 2.0 / B)
    if _want_results:
        return loss, res
    return loss


# revision 10
# speedup vs baseline: 1.2169x; 1.1327x over previous
"""Trainium2 Bass kernel for nn_CustomLoss (argmax-distance weighted loss).

reference:
    arg = argmax(target, axis=1)              # [B]
    delta = distance[arg]                     # [B]
    err = |distance[None,:] - delta[:,None]| + 1
    loss = sum((output - target) * err) / B

Algorithm (no gathers, data-parallel over 8 NeuronCores):
  With dist = [-0.5, -0.34, 0, 0.34, 0.5] and e_a = [t_a >= max_c t_c]:
    2*delta = (e4 - e0) + 0.68*(e3 - e1)          (dist[2]=0 -> e2 unused;
                                                   argmax==2 gives w2=0=2*dist[2])
    err[b,c] + 1 -> (|2*delta - 2*dist_c| + 2)/2
  loss*2*B = sum over b,c of (o - t) * (|w2 - 2*dist_c| + 2)

Per-core layout: rows on 128 partitions, 5 classes interleaved along free dim,
8 tiles of [128, 2560].  Engines: DMA loads t (f32, HWDGE) + o (bf16 cast,
SWDGE); GPSIMD does the 4-op max tree; VectorE does compares and the fused
(w+2)*d product with per-partition accum; ScalarE does the bf16 cast and the
5 Abs activations.  Output: [128, ntiles] partial sums per core, reduced on
host and divided by 2*B.
"""

from contextlib import ExitStack

import numpy as np

P = 128
C = 5
DIST = (-0.5, -0.34, 0.0, 0.34, 0.5)
B = 4194304
NCORES = 8
ROWS_PER_CORE = B // NCORES  # 524288
G = 512                      # rows per partition per tile
NTILES = ROWS_PER_CORE // (P * G)  # 8

_CACHE = {}


def _build_nc():
    import concourse.bacc as bacc
    import concourse.mybir as mybir
    import concourse.tile as tile

    F32 = mybir.dt.float32
    BF16 = mybir.dt.bfloat16
    FREE = C * G

    nc = bacc.Bacc(target_bir_lowering=False)

    # Register activation-bias constants (-2*dist[c]) in the const-AP database,
    # mirroring what Bass.__init__ does for 0.0/1.0.
    for c in range(C):
        val = -2.0 * DIST[c]
        if (F32, val) not in nc.const_aps.aps:
            tensor = nc.alloc_sbuf_tensor(f"const-f32-bias{c}", [P, 1], F32)
            nc.gpsimd.memset(tensor.ap(), val)
            nc.const_aps.aps[(F32, val)] = tensor.ap()
    nc.all_engine_barrier()

    t_in = nc.declare_dram_parameter("t", [ROWS_PER_CORE, C], F32, isOutput=False)
    o_in = nc.declare_dram_parameter("o", [ROWS_PER_CORE, C], F32, isOutput=False)
    out = nc.declare_dram_parameter("out", [1, 2 * G], F32, isOutput=True)

    # row = n*(P*G) + p*G + g ; per-partition data is contiguous in DRAM
    t_tiled = t_in.rearrange("(n p g) c -> n p (g c)", p=P, g=G)
    o_tiled = o_in.rearrange("(n p g) c -> n p (g c)", p=P, g=G)

    ones_bf16 = nc.const_aps.aps[(BF16, 1.0)]  # [128, 1] of 1.0, preregistered

    with ExitStack() as ctx:
        tc = ctx.enter_context(tile.TileContext(nc))
        pool = ctx.enter_context(tc.tile_pool(name="work", bufs=3))
        psp = ctx.enter_context(tc.tile_pool(name="ps", bufs=1, space="PSUM"))
        outp = ctx.enter_context(tc.tile_pool(name="outp", bufs=1))
        ps_p = psp.tile([1, G], F32)   # sum of wI*d
        ps_d = psp.tile([1, G], F32)   # sum of d

        # Software-pipelined emission: tile k's front work (loads, cast, max,
        # compares) is emitted before tile k-1's back work (Abs weights,
        # product, matmuls) so each engine's in-order stream has cross-tile
        # lookahead and DVE never stalls on ScalarE's Abs chain.
        state = {}

        def emit_front(k):
            t = pool.tile([P, FREE], F32, tag="t", name="t", bufs=3)
            nc.sync.dma_start(t[:, :], t_tiled[k])
            o = pool.tile([P, FREE], BF16, tag="o", name="o", bufs=3)
            nc.gpsimd.dma_start(o[:, :], o_tiled[k])  # f32 -> bf16 cast in DMA

            tb = pool.tile([P, FREE], BF16, tag="tb", name="tb", bufs=3)
            nc.scalar.copy(tb[:, :], t[:, :])  # ACT cast f32->bf16

            tv = t[:, :].rearrange("p (g c) -> p g c", c=C)

            # m[p,g] = max over the 5 classes (segmented reduce, unit-stride)
            m = pool.tile([P, G], F32, tag="m", name="m", bufs=4)
            nc.vector.tensor_reduce(
                m[:, :], tv, axis=mybir.AxisListType.X, op=mybir.AluOpType.max
            )

            # E[p,g,c] = [t >= m]  (one pass, m broadcast along class dim)
            E = pool.tile([P, FREE], BF16, tag="E", name="E", bufs=3)
            nc.vector.tensor_tensor(
                E[:, :].rearrange("p (g c) -> p g c", c=C),
                tv,
                m[:, :].to_broadcast([P, G, C]),
                op=mybir.AluOpType.is_ge,
            )

            Ev = E[:, :].rearrange("p (g c) -> p g c", c=C)
            u = pool.tile([P, G], BF16, tag="u", name="u", bufs=4)
            nc.vector.tensor_sub(u[:, :], Ev[:, :, 4], Ev[:, :, 0])
            v = pool.tile([P, G], BF16, tag="v", name="v", bufs=4)
            nc.vector.tensor_sub(v[:, :], Ev[:, :, 3], Ev[:, :, 1])
            w2 = pool.tile([P, G], BF16, tag="w2", name="w2", bufs=4)
            # w2 = (v * 0.68) + u  == 2*delta
            nc.vector.scalar_tensor_tensor(
                w2[:, :], v[:, :], 0.68, u[:, :],
                mybir.AluOpType.mult, mybir.AluOpType.add,
            )

            d = pool.tile([P, FREE], BF16, tag="d", name="d", bufs=4)
            nc.vector.tensor_sub(d[:, :], o[:, :], tb[:, :])
            state[k] = (w2, d)

        def emit_back(k):
            w2, d = state.pop(k)
            # wI[:, g, c] = |w2 - 2*dist[c]|  (ScalarE)
            wI = pool.tile([P, FREE], BF16, tag="wI", name="wI", bufs=3)
            wIv = wI[:, :].rearrange("p (g c) -> p g c", c=C)
            for c in range(C):
                nc.scalar.activation(
                    wIv[:, :, c], w2[:, :], mybir.ActivationFunctionType.Abs,
                    bias=-2.0 * DIST[c], scale=1.0,
                )

            # p = wI * d  (bf16 2x mode), then TensorE ones-matmul reduces
            # p and d into PSUM accumulators across all tiles
            p = pool.tile([P, FREE], BF16, tag="p", name="p", bufs=3)
            nc.vector.tensor_mul(p[:, :], wI[:, :], d[:, :])
            for j in range(C):
                first = k == 0 and j == 0
                last = k == NTILES - 1 and j == C - 1
                nc.tensor.matmul(
                    ps_p[:, :], ones_bf16, p[:, j * G : (j + 1) * G],
                    start=first, stop=last,
                )
                nc.tensor.matmul(
                    ps_d[:, :], ones_bf16, d[:, j * G : (j + 1) * G],
                    start=first, stop=last,
                )

        for k in range(NTILES):
            emit_front(k)
            if k >= 1:
                emit_back(k - 1)
        emit_back(NTILES - 1)

        # readout: [1, 2G] f32 -> DRAM; host computes (sum0 + 2*sum1) / 2B
        res = outp.tile([1, 2 * G], F32)
        nc.scalar.copy(res[:, 0:G], ps_p[:, :])
        nc.scalar.copy(res[:, G : 2 * G], ps_d[:, :])
        nc.sync.dma_start(out[:, :], res[:, :])
    nc.finalize()
    return nc


def _get_nc():
    if "nc" not in _CACHE:
        _CACHE["nc"] = _build_nc()
    return _CACHE["nc"]


def kernel(output, target, distance, _want_results=False):
    from concourse.bass_utils import run_bass_kernel_spmd

    output = np.asarray(output, dtype=np.float32)
    target = np.asarray(target, dtype=np.float32)
    distance = np.asarray(distance, dtype=np.float32)
    assert output.shape == (B, C) and target.shape == (B, C)
    assert np.allclose(distance, np.asarray(DIST, np.float32)), distance

    nc = _get_nc()
    o_sh = output.reshape(NCORES, ROWS_PER_CORE, C)
    t_sh = target.reshape(NCORES, ROWS_PER_CORE, C)
    in_maps = [
        {"t": np.ascontiguousarray(t_sh[i]), "o": np.ascontiguousarray(o_sh[i])}
        for i in range(NCORES)
    ]
    res = run_bass_kernel_spmd(nc, in_maps, core_ids=list(range(NCORES)))
    total = 0.0
    for r in res.results:
        arr = r["out"].astype(np.float64).reshape(2, G)
        total += float(arr[0].sum() + 2.0 * arr[1].sum())
    loss = np.float32(total / 2.0 / B)
    if _want_results:
        return loss, res
    return loss
